# revision 1
# baseline (speedup 1.0000x reference)
"""3D window attention (B=32, N=513, D=768, H=12) on 8 trn2 NeuronCores. v4

Data-parallel over batch (4 per core). bf16 datapath + fp8-DoubleRow
bias injection:
  A:  x(bf16) -> xT via PE transposes
  B2: q[g] = [128(2 heads), T] bf16, k[g] = [128, 4*640] bf16, in SBUF
      (no DRAM spills)
  B:  v[b] = [128, 6 key-tiles, 12*(64+1)] bf16 with ones column
  C:  per b, per g: S = bias(fp8 DoubleRow, half PE cost) + K^T Q (bf16),
      exp(S*0.125) on ACT -> bf16 P, AV + denominator ride-along,
      normalize from PSUM. stail/Ut use the S-pool psum tile.
  D:  y = a @ Wp^T + b_proj in fp32r, interleaved per-batch into C so
      its PE work fills the ACT-bound exp window.
"""

import numpy as np

B, N_TOK, DIM, HEADS = 32, 513, 768, 12
HD = DIM // HEADS             # 64
N_CORES = 8
B_PER = B // N_CORES          # 4
T = B_PER * N_TOK             # 2052
G = HEADS // 2                # 6 head pairs
MT = 5                        # key tiles per batch (513 -> 4*128 + 1)
ES = 0.125                    # exp scale (= head_dim**-0.5)
BPK = 8.0                     # bias prepack = 1/ES
MASK = -240.0                 # fp8 mask -> exp(-30) ~ 9e-14

_CACHE = {}


def _build_nc():
    import concourse.bacc as bacc
    import concourse.mybir as mybir
    import concourse.tile as tile
    from concourse.ap import AP

    F32 = mybir.dt.float32
    F32R = mybir.dt.float32r
    BF16 = mybir.dt.bfloat16
    FP8 = mybir.dt.float8e4
    AF = mybir.ActivationFunctionType
    DR = mybir.MatmulPerfMode.DoubleRow

    nc = bacc.Bacc("TRN2", target_bir_lowering=False, debug=False)

    x_d = nc.dram_tensor("xb", [T, DIM], BF16, kind="ExternalInput")
    wqk_d = nc.dram_tensor("wqkT", [DIM, 2 * DIM], BF16, kind="ExternalInput")
    wv_d = nc.dram_tensor("wvT", [DIM, DIM], BF16, kind="ExternalInput")
    bt8_d = nc.dram_tensor("bt8", [G, 128, 2 * MT * N_TOK], FP8,
                           kind="ExternalInput")
    ebt_d = nc.dram_tensor("ebt", [128, G * 10], F32, kind="ExternalInput")
    id16_d = nc.dram_tensor("id16", [128, 128], BF16, kind="ExternalInput")
    id0_d = nc.dram_tensor("id0", [128, 256], FP8, kind="ExternalInput")
    wp_d = nc.dram_tensor("wpT", [DIM, DIM], BF16, kind="ExternalInput")
    bb_d = nc.dram_tensor("bb", [128, DIM], F32, kind="ExternalInput")
    y_d = nc.dram_tensor("y", [T, DIM], F32, kind="ExternalOutput")

    t_tiles = [(i * 128, min(128, T - i * 128)) for i in range((T + 127) // 128)]

    with tile.TileContext(nc) as tc:
        consts_cm = tc.tile_pool(name="consts", bufs=1)
        consts = consts_cm.__enter__()
        pers_cm = tc.tile_pool(name="pers", bufs=1)
        pers = pers_cm.__enter__()
        xTp_cm = tc.tile_pool(name="xTp", bufs=1)
        xTp = xTp_cm.__enter__()

        id16 = consts.tile([128, 128], BF16, tag="id16", name="id16")
        id0 = consts.tile([128, 256], FP8, tag="id0", name="id0")
        bb = consts.tile([128, DIM], F32, tag="bb", name="bb")
        ebt = consts.tile([128, G * 10], F32, tag="ebt", name="ebt")
        nc.sync.dma_start(id16[:], id16_d.ap())
        nc.sync.dma_start(id0[:], id0_d.ap())
        nc.sync.dma_start(bb[:], bb_d.ap())
        nc.sync.dma_start(ebt[:], ebt_d.ap())
        id0p = id0.rearrange("p (i c) -> p i c", c=128)

        qS = [pers.tile([128, T], BF16, tag=f"q{g}", name=f"q{g}")
              for g in range(G)]
        kS = [pers.tile([128, T + 127], BF16, tag=f"k{g}", name=f"k{g}")
              for g in range(G)]
        btS = [pers.tile([128, 2 * MT * N_TOK], FP8, tag=f"bt{g}",
                         name=f"bt{g}") for g in range(G)]
        v8 = [pers.tile([128, 6, HEADS * (HD + 1)], BF16, tag=f"v{b}",
                        name=f"v{b}") for b in range(B_PER)]
        aT = None  # allocated after B2 when wqk space frees
        wvp_cm = tc.tile_pool(name="wvp", bufs=1)
        wvp = wvp_cm.__enter__()
        wqp_cm = tc.tile_pool(name="wqp", bufs=1)
        wqp = wqp_cm.__enter__()
        wqk = wqp.tile([128, 6, 2 * DIM], BF16, tag="wqk", name="wqk")
        wv = wvp.tile([128, 6, DIM], BF16, tag="wv", name="wv")
        wqk_src = wqk_d.ap().rearrange("(c p) d -> p c d", p=128)
        wv_src = wv_d.ap().rearrange("(c p) d -> p c d", p=128)

        for b in range(B_PER):
            nc.gpsimd.memset(v8[b][:, 4:6, :], 0.0)
        for b in range(B_PER):
            v3 = v8[b].rearrange("p m (h x) -> p m h x", x=HD + 1)
            nc.gpsimd.memset(v3[:, :, :, HD], 1.0)

        xT = xTp.tile([128, 6, T], BF16, tag="xT", name="xT")

        # ---------------- Phase A: xT = transpose(x) ----------------
        pA_sb = tc.tile_pool(name="xload", bufs=3)
        pA_ps = tc.tile_pool(name="psA", bufs=8, space="PSUM")
        with pA_sb as xload, pA_ps as psA:
            for tp in range(5):
                if tp == 2:
                    for ci in range(6):
                        nc.sync.dma_start(wqk[:, ci], wqk_src[:, ci])
                elif tp == 4:
                    for ci in range(6):
                        nc.sync.dma_start(wv[:, ci], wv_src[:, ci])
                p0 = tp * 512
                nsub = 4 if tp < 4 else 1
                xin = xload.tile([128, 4, DIM], BF16, tag="xin", name="xin")
                if nsub == 4:
                    nc.scalar.dma_start(
                        xin[:],
                        x_d.ap()[p0:p0 + 512].rearrange("(f p) d -> p f d",
                                                        p=128))
                else:
                    nc.scalar.dma_start(xin[0:4, 0, :],
                                        x_d.ap()[2048:2052, :])
                for sub in range(nsub):
                    ti = tp * 4 + sub
                    t0, ts = t_tiles[ti]
                    for ci in range(6):
                        pt = psA.tile([128, 128], BF16, tag="pA", name="pA")
                        nc.tensor.transpose(
                            pt[:, :ts],
                            xin[:ts, sub, ci * 128:(ci + 1) * 128],
                            id16[:ts, :ts],
                        )
                        if (ti * 6 + ci) % 2 == 0:
                            nc.vector.tensor_copy(xT[:, ci, t0:t0 + ts],
                                                  pt[:, :ts])
                        else:
                            nc.scalar.copy(xT[:, ci, t0:t0 + ts], pt[:, :ts])

        # ---------------- Phase B2: q / k in SBUF ----------------
        pB2_ps = tc.tile_pool(name="psB2", bufs=7, space="PSUM")
        pB2_pt = tc.tile_pool(name="psB2t", bufs=1, space="PSUM")
        with pB2_ps as psB2, pB2_pt as psB2t:
            cast_i = 0

            def cast(out, in_):
                nonlocal cast_i
                cast_i += 1
                if cast_i % 2 == 0:
                    nc.vector.tensor_copy(out, in_)
                else:
                    nc.scalar.copy(out, in_)

            for g in range(G):
                # q: both heads' rows, token-chunks of 512 (+ 4-col tail)
                for c4 in range(4):
                    psq = psB2.tile([128, 512], F32, tag="psq", name="psq")
                    for ci in range(6):
                        nc.tensor.matmul(
                            psq[:], wqk[:, ci, g * 128:(g + 1) * 128],
                            xT[:, ci, c4 * 512:c4 * 512 + 512],
                            start=(ci == 0), stop=(ci == 5))
                    cast(qS[g][:, c4 * 512:c4 * 512 + 512], psq[:])
                psqt = psB2t.tile([128, 4], F32, tag="psqt", name="psqt")
                for ci in range(6):
                    nc.tensor.matmul(
                        psqt[:], wqk[:, ci, g * 128:(g + 1) * 128],
                        xT[:, ci, 2048:2052],
                        start=(ci == 0), stop=(ci == 5))
                cast(qS[g][:, 2048:2052], psqt[:])
                # k: per-batch 640-blocks
                for b in range(B_PER):
                    psk = psB2.tile([128, 512], F32, tag="psq", name="psk")
                    for ci in range(6):
                        nc.tensor.matmul(
                            psk[:], wqk[:, ci, DIM + g * 128:DIM + (g + 1) * 128],
                            xT[:, ci, b * N_TOK:b * N_TOK + 512],
                            start=(ci == 0), stop=(ci == 5))
                    cast(kS[g][:, b * N_TOK:b * N_TOK + 512], psk[:])
                psk4 = psB2t.tile([128, 4], F32, tag="psqt", name="psk4")
                for ci in range(6):
                    nc.tensor.matmul(
                        psk4[:], wqk[:, ci, DIM + g * 128:DIM + (g + 1) * 128],
                        xT.rearrange("p c (b n) -> p c b n", n=N_TOK)
                          [:, ci, :, 512],
                        start=(ci == 0), stop=(ci == 5))
                cast(kS[g][:, 0:B_PER * N_TOK].rearrange(
                         "p (b w) -> p b w", w=N_TOK)[:, :, 512],
                     psk4[:])
                nc.gpsimd.memset(kS[g][:, T:T + 127], 0.0)

        wqp_cm.__exit__(None, None, None)

        # ---------------- Phase B: v (per-batch key-tile layout) ---------
        pB_ps = tc.tile_pool(name="psB", bufs=3, space="PSUM")
        pB_pt = tc.tile_pool(name="psBt", bufs=1, space="PSUM")
        with pB_ps as psB, pB_pt as psBt:
            for g in range(G):
                eng = (nc.gpsimd, nc.sync, nc.scalar)[g % 3]
                eng.dma_start(btS[g][:], bt8_d.ap()[g])
            for b in range(B_PER):
                v3 = v8[b].rearrange("p m (h x) -> p m h x", x=HD + 1)
                for mt in range(4):
                    psv = psB.tile([128, DIM], F32, tag="psv", name="psv")
                    lhs0 = b * N_TOK + mt * 128
                    for ci in range(6):
                        lhsT = xT[:, ci, lhs0:lhs0 + 128]
                        nc.tensor.matmul(psv[:, 0:512], lhsT,
                                         wv[:, ci, 0:512],
                                         start=(ci == 0), stop=(ci == 5))
                        nc.tensor.matmul(psv[:, 512:768], lhsT,
                                         wv[:, ci, 512:768],
                                         start=(ci == 0), stop=(ci == 5))
                    src = psv[:].rearrange("p (h d) -> p h d", d=HD)
                    if b % 2 == 0:
                        nc.vector.tensor_copy(v3[:, mt, :, 0:HD], src)
                    else:
                        nc.scalar.copy(v3[:, mt, :, 0:HD], src)
            psvt = psBt.tile([128, DIM], F32, tag="psvt", name="psvt")
            for ci in range(6):
                lhsT = xT.rearrange("p c (b n) -> p c b n", n=N_TOK)[:, ci, :, 512]
                nc.tensor.matmul(psvt[:B_PER, 0:512], lhsT, wv[:, ci, 0:512],
                                 start=(ci == 0), stop=(ci == 5))
                nc.tensor.matmul(psvt[:B_PER, 512:768], lhsT, wv[:, ci, 512:768],
                                 start=(ci == 0), stop=(ci == 5))
            vst = pers.tile([B_PER, DIM], BF16, tag="vst", name="vst")
            nc.vector.tensor_copy(vst[:], psvt[:B_PER, :])
            for b in range(B_PER):
                v3 = v8[b].rearrange("p m (h x) -> p m h x", x=HD + 1)
                nc.sync.dma_start(
                    v3[0:1, 4, :, 0:HD],
                    vst.rearrange("p (h d) -> p h d", d=HD)[b:b + 1])

        wvp_cm.__exit__(None, None, None)
        xTp_cm.__exit__(None, None, None)

        # ---------------- Phase C + interleaved D ----------------
        aTp_cm = tc.tile_pool(name="aTp", bufs=1)
        aTp = aTp_cm.__enter__()
        aT = [aTp.tile([128, T], BF16, tag=f"aT{g}", name=f"aT{g}")
              for g in range(G)]
        wpal = aTp.tile([128, 6, DIM], BF16, tag="wpal", name="wpal")
        wp_src = wp_d.ap().rearrange("(c p) d -> p c d", p=128)
        for ci in range(6):
            nc.scalar.dma_start(wpal[:, ci], wp_src[:, ci])
        pC_pm = tc.tile_pool(name="pmp", bufs=5)
        pC_pt = tc.tile_pool(name="ptp", bufs=3)
        pC_sm = tc.tile_pool(name="smallp", bufs=3)
        pD3 = tc.tile_pool(name="ytp", bufs=4)
        pS_ps = tc.tile_pool(name="psS", bufs=2, space="PSUM")
        pU_ps = tc.tile_pool(name="psU", bufs=2, space="PSUM")
        pT_ps = tc.tile_pool(name="psT", bufs=1, space="PSUM")
        pD_ps = tc.tile_pool(name="psD", bufs=1, space="PSUM")
        d_done = [0]
        d_half = [0]
        d_yt = [None]
        d_avail = [0]
        with pC_pm as pmp, pC_pt as ptp, pC_sm as smallp, pD3 as ytp, \
             pS_ps as psS, pU_ps as psU, pT_ps as psT, pD_ps as psD:
            dpool = [psD]

            def emit_d_pass():
                # one 6-matmul pass (half an output tile) per call; calls are
                # spaced by C work so the single psD buffer never stalls PE
                if d_done[0] >= len(t_tiles):
                    return False
                t0, ts = t_tiles[d_done[0]]
                if t0 + ts > d_avail[0]:
                    return False
                half = d_half[0]
                (c0, cw) = ((0, 512), (512, 256))[half]
                psy = dpool[0].tile([128, 512], F32, tag="psy",
                                    name="psy")
                for j in range(6):
                    nc.tensor.matmul(psy[:ts, 0:cw], aT[j][:, t0:t0 + ts],
                                     wpal[:, j, c0:c0 + cw],
                                     start=(j == 0), stop=(j == 5))
                if half == 0:
                    yt = ytp.tile([128, DIM], F32, tag="yt", name="yt")
                    d_yt[0] = yt
                    nc.vector.tensor_add(yt[:ts, 0:512],
                                         psy[:ts, 0:512], bb[:ts, 0:512])
                    d_half[0] = 1
                else:
                    yt = d_yt[0]
                    nc.vector.tensor_add(yt[:ts, 512:768],
                                         psy[:ts, 0:256], bb[:ts, 512:768])
                    nc.sync.dma_start(y_d.ap()[t0:t0 + ts, :], yt[:ts, :])
                    d_half[0] = 0
                    d_done[0] += 1
                return True

            for b in range(B_PER):
                for g in range(G):
                    emit_d_pass()
                    q, k, bt = qS[g], kS[g], btS[g]
                    btp = bt[:]
                    bt_pitch = btp.ap[0][0]

                    U2 = [psU.tile([HD + 1, 512], F32, tag="U", name="U")
                          for _ in range(2)]
                    pms = {}
                    prev = None

                    def emit_av(mt, pm):
                        st = (mt == 0)
                        sp = (mt == MT - 1)
                        for h2 in range(2):
                            h = 2 * g + h2
                            nc.tensor.matmul(
                                U2[h2][:],
                                v8[b][:, mt, h * 65:h * 65 + 65],
                                pm[:, h2 * 512:h2 * 512 + 512],
                                start=st, stop=sp)

                    for mt in range(MT):
                        sm = psS.tile([128, 1024], F32, tag="S", name="S")
                        for h2 in range(2):
                            c0 = h2 * 512
                            # fp8 DoubleRow bias inject: (I|0) x (bt|bt)
                            nc.tensor.matmul(
                                sm[:, c0:c0 + 512], id0p[:],
                                AP(btp.tensor,
                                   (h2 * MT + mt) * N_TOK,
                                   [[bt_pitch, 128], [0, 2], [1, 512]]),
                                start=True, stop=False, perf_mode=DR)
                            nc.tensor.matmul(
                                sm[:, c0:c0 + 512],
                                k[h2 * HD:h2 * HD + HD,
                                  b * N_TOK + mt * 128:
                                  b * N_TOK + mt * 128 + 128],
                                q[h2 * HD:h2 * HD + HD,
                                  b * N_TOK:b * N_TOK + 512],
                                start=False, stop=True)
                        pm = pmp.tile([128, 1024], BF16, tag="pm", name="pm")
                        nc.scalar.activation(pm[:], sm[:], AF.Exp, bias=0.0,
                                             scale=ES)
                        if prev is not None:
                            emit_av(*prev)
                        prev = (mt, pm)
                    emit_av(*prev)

                    # tail-query logits into an S-pool tile (cols 0:10),
                    # tail AV/denominator into cols 512:514.
                    stail = psT.tile([128, 12], F32, tag="st", name="st")
                    for mt in range(MT):
                        for h2 in range(2):
                            nc.tensor.matmul(
                                stail[:, 2 * mt + h2:2 * mt + h2 + 1],
                                k[h2 * HD:h2 * HD + HD,
                                  b * N_TOK + mt * 128:
                                  b * N_TOK + mt * 128 + 128],
                                q[h2 * HD:h2 * HD + HD,
                                  b * N_TOK + 512:b * N_TOK + 513],
                                start=True, stop=True)
                    praw = smallp.tile([128, 10], F32, tag="praw", name="praw")
                    nc.scalar.activation(praw[:], stail[:, 0:10], AF.Exp,
                                         bias=0.0, scale=ES)
                    ptail = ptp.tile([128, 10], BF16, tag="pt", name="pt")
                    nc.vector.tensor_mul(ptail[:], praw[:],
                                         ebt[:, g * 10:g * 10 + 10])
                    for h2 in range(2):
                        h = 2 * g + h2
                        for mt in range(MT):
                            nc.tensor.matmul(
                                stail[0:HD + 1, 10 + h2:11 + h2],
                                v8[b][:, mt, h * 65:h * 65 + 65],
                                ptail[:, 2 * mt + h2:2 * mt + h2 + 1],
                                start=(mt == 0), stop=(mt == MT - 1))
                    emit_d_pass()
                    # normalize: tail first so the stail psum frees early
                    rnt = smallp.tile([1, 2], F32, tag="rnt", name="rnt")
                    nc.vector.reciprocal(rnt[:], stail[HD:HD + 1, 10:12])
                    for h2 in range(2):
                        bct = smallp.tile([HD, 1], F32, tag="bct", name="bct")
                        nc.gpsimd.partition_broadcast(bct[:],
                                                      rnt[:, h2:h2 + 1])
                        nc.vector.tensor_mul(
                            aT[g][h2 * HD:h2 * HD + HD,
                                  b * N_TOK + 512:b * N_TOK + 513],
                            stail[0:HD, 10 + h2:11 + h2], bct[:])
                    for h2 in range(2):
                        rn = smallp.tile([1, 512], F32, tag="rn", name="rn")
                        nc.vector.reciprocal(rn[:], U2[h2][HD:HD + 1, :])
                        bc = smallp.tile([HD, 512], F32, tag="bc", name="bc")
                        nc.gpsimd.partition_broadcast(bc[:], rn[:])
                        nc.vector.tensor_mul(
                            aT[g][h2 * HD:h2 * HD + HD,
                                  b * N_TOK:b * N_TOK + 512],
                            U2[h2][0:HD, :], bc[:])

                d_avail[0] = (b + 1) * N_TOK
            while emit_d_pass():
                pass

        aTp_cm.__exit__(None, None, None)
        pers_cm.__exit__(None, None, None)
        consts_cm.__exit__(None, None, None)

    nc.compile()
    return nc


def get_nc():
    if "nc" not in _CACHE:
        _CACHE["nc"] = _build_nc()
    return _CACHE["nc"]


def host_prep(w_qkv, bias_table, w_proj, b_proj, rel_index):
    """Host-side packing shared by all cores."""
    import ml_dtypes
    E4 = ml_dtypes.float8_e4m3
    BF = ml_dtypes.bfloat16
    w = np.asarray(w_qkv, dtype=np.float32)
    wqkT = np.ascontiguousarray(w[0:2 * DIM].T).astype(BF)     # [c, 1536]
    wvT = np.ascontiguousarray(w[2 * DIM:3 * DIM].T).astype(BF)

    tbl = np.asarray(bias_table, dtype=np.float32)
    gat = tbl[np.asarray(rel_index)]            # [n(query), m(key), h]
    Bm = gat.transpose(2, 1, 0)                 # [h, m(key), n(query)]
    bt = np.full((G, 2, MT, 128, N_TOK), MASK, dtype=np.float32)
    for g in range(G):
        for h2 in range(2):
            for mt in range(MT):
                m0 = mt * 128
                ml = min(128, N_TOK - m0)
                bt[g, h2, mt, 0:ml, :] = BPK * Bm[2 * g + h2, m0:m0 + ml, :]
    bt8 = np.clip(bt, -240, 240).astype(E4)
    bt8 = np.ascontiguousarray(bt8.transpose(0, 3, 1, 2, 4)).reshape(
        G, 128, 2 * MT * N_TOK)

    # tail-query exp(bias) factors: ebt[k, g*10 + 2*mt + h2], 0 for pad keys
    ebt = np.zeros((128, G * 10), dtype=np.float32)
    with np.errstate(under="ignore"):
        for g in range(G):
            for mt in range(MT):
                m0 = mt * 128
                ml = min(128, N_TOK - m0)
                for h2 in range(2):
                    ebt[0:ml, g * 10 + 2 * mt + h2] = np.exp(
                        Bm[2 * g + h2, m0:m0 + ml, 512])

    id16 = np.eye(128, dtype=np.float32).astype(BF)
    id0 = np.zeros((128, 256), dtype=np.float32)
    id0[:, 0:128] = np.eye(128, dtype=np.float32)
    id0 = id0.astype(E4)
    wpT = np.ascontiguousarray(np.asarray(w_proj, dtype=np.float32).T).astype(BF)
    bb = np.ascontiguousarray(
        np.broadcast_to(np.asarray(b_proj, dtype=np.float32), (128, DIM)))
    return {"wqkT": wqkT, "wvT": wvT, "bt8": bt8, "ebt": ebt, "id16": id16,
            "id0": id0, "wpT": wpT, "bb": bb}


def prep_x(x_core):
    import ml_dtypes
    return np.asarray(x_core, dtype=np.float32).reshape(T, DIM).astype(
        ml_dtypes.bfloat16)


def kernel(x, w_qkv, bias_table, w_proj, b_proj, rel_index):
    import time
    from concourse.bass_utils import run_bass_kernel_spmd

    x = np.asarray(x, dtype=np.float32)
    shared = host_prep(w_qkv, bias_table, w_proj, b_proj, rel_index)
    nc = get_nc()
    in_maps = []
    for c in range(N_CORES):
        m = {"xb": prep_x(x[c * B_PER:(c + 1) * B_PER])}
        m.update(shared)
        in_maps.append(m)
    # Transient NRT_EXEC_UNIT_UNRECOVERABLE failures have been observed on
    # this fabric; an identical retry passes, so guard the execution.
    last_exc = None
    for attempt in range(3):
        try:
            res = run_bass_kernel_spmd(nc, in_maps, core_ids=list(range(N_CORES)))
            break
        except Exception as e:
            last_exc = e
            time.sleep(2.0)
    else:
        raise last_exc
    out = np.concatenate(
        [res.results[c]["y"].reshape(B_PER, N_TOK, DIM) for c in range(N_CORES)],
        axis=0,
    )
    return out



# revision 5
# speedup vs baseline: 1.1063x; 1.1063x over previous
"""3D window attention (B=32, N=513, D=768, H=12) on 8 trn2 NeuronCores. v5

Data-parallel over batch (4 per core). bf16 datapath:
  host: x pre-transposed to xT [128, 6ci, T]; bias gathered + exponentiated
        to bf16 EB tables (exp(S+B) = exp(S)*exp(B)) -> no PE bias inject.
  B2:   q[g] = [128(2 heads), T] bf16, k[g] = [128, T] bf16, in SBUF
  B:    v[b] = [128, 5 key-tiles, 12*(64+1)] bf16 with ones column
  C:    per (b,g): S = K^T Q (start/stop, no inject), exp(S*0.125) on ACT,
        pmb = pm * EB on DVE (4x bf16 mode), AV + denominator ride-along.
        key 512 handled as a rank-1 (K=1) row path; tail query (512) ops
        deferred one iteration so PE never waits on the ACT->DVE round trip.
  D:    y = a @ Wp^T + b_proj interleaved per-batch into C.
"""

import numpy as np

B, N_TOK, DIM, HEADS = 32, 513, 768, 12
HD = DIM // HEADS             # 64
N_CORES = 8
B_PER = B // N_CORES          # 4
T = B_PER * N_TOK             # 2052
G = HEADS // 2                # 6 head pairs
ES = 0.125                    # exp scale (= head_dim**-0.5)

_CACHE = {}


def _build_nc():
    import concourse.bacc as bacc
    import concourse.mybir as mybir
    import concourse.tile as tile

    F32 = mybir.dt.float32
    BF16 = mybir.dt.bfloat16
    AF = mybir.ActivationFunctionType

    nc = bacc.Bacc("TRN2", target_bir_lowering=False, debug=False)

    xT_d = nc.dram_tensor("xT", [128, 6 * T], BF16, kind="ExternalInput")
    wqk_d = nc.dram_tensor("wqkT", [DIM, 2 * DIM], BF16, kind="ExternalInput")
    wv_d = nc.dram_tensor("wvT", [DIM, DIM], BF16, kind="ExternalInput")
    eb_d = nc.dram_tensor("eb", [G, 128, 4 * 1024], BF16, kind="ExternalInput")
    eb4_d = nc.dram_tensor("eb4", [1, G * 1024], BF16, kind="ExternalInput")
    ebt_d = nc.dram_tensor("ebt", [128, G * 8 + 16], F32, kind="ExternalInput")
    wp_d = nc.dram_tensor("wpT", [DIM, DIM], BF16, kind="ExternalInput")
    bb_d = nc.dram_tensor("bb", [128, DIM], F32, kind="ExternalInput")
    y_d = nc.dram_tensor("y", [T, DIM], F32, kind="ExternalOutput")

    t_tiles = [(i * 128, min(128, T - i * 128)) for i in range((T + 127) // 128)]

    with tile.TileContext(nc) as tc:
        consts_cm = tc.tile_pool(name="consts", bufs=1)
        consts = consts_cm.__enter__()
        pers_cm = tc.tile_pool(name="pers", bufs=1)
        pers = pers_cm.__enter__()
        xTp_cm = tc.tile_pool(name="xTp", bufs=1)
        xTp = xTp_cm.__enter__()

        bb = consts.tile([128, DIM], F32, tag="bb", name="bb")
        ebt = consts.tile([128, G * 8 + 16], F32, tag="ebt", name="ebt")
        eb4t = consts.tile([1, G * 1024], BF16, tag="eb4", name="eb4")

        qS = [pers.tile([128, T], BF16, tag=f"q{g}", name=f"q{g}")
              for g in range(G)]
        kS = [pers.tile([128, T], BF16, tag=f"k{g}", name=f"k{g}")
              for g in range(G)]
        ebS = [pers.tile([128, 4, 1024], BF16, tag=f"eb{g}", name=f"eb{g}")
               for g in range(G)]
        v8 = [pers.tile([128, 5, HEADS * (HD + 1)], BF16, tag=f"v{b}",
                        name=f"v{b}") for b in range(B_PER)]
        wvp_cm = tc.tile_pool(name="wvp", bufs=1)
        wvp = wvp_cm.__enter__()
        wqp_cm = tc.tile_pool(name="wqp", bufs=1)
        wqp = wqp_cm.__enter__()
        wqk = wqp.tile([128, 6, 2 * DIM], BF16, tag="wqk", name="wqk")
        wv = wvp.tile([128, 6, DIM], BF16, tag="wv", name="wv")
        xT = xTp.tile([128, 6, T], BF16, tag="xT", name="xT")

        wqk_src = wqk_d.ap().rearrange("(c p) d -> p c d", p=128)
        wv_src = wv_d.ap().rearrange("(c p) d -> p c d", p=128)
        xT_src = xT_d.ap().rearrange("p (c t) -> p c t", t=T)

        # input DMAs, ordered so B2 unblocks earliest: (wqk, xT) interleaved
        # per-ci, then wv, then the EB tables.
        for ci in range(6):
            nc.sync.dma_start(wqk[:, ci], wqk_src[:, ci])
            nc.sync.dma_start(xT[:, ci], xT_src[:, ci])
        for ci in range(6):
            nc.sync.dma_start(wv[:, ci], wv_src[:, ci])
        for g in range(G):
            nc.sync.dma_start(ebS[g][:], eb_d.ap()[g])
        nc.sync.dma_start(bb[:], bb_d.ap())
        nc.sync.dma_start(ebt[:], ebt_d.ap())
        nc.sync.dma_start(eb4t[:], eb4_d.ap())

        for b in range(B_PER):
            v3 = v8[b].rearrange("p m (h x) -> p m h x", x=HD + 1)
            nc.gpsimd.memset(v3[:, :, :, HD], 1.0)

        # ---------------- Phase B2: q / k in SBUF ----------------
        pB2_ps = tc.tile_pool(name="psB2", bufs=7, space="PSUM")
        pB2_pt = tc.tile_pool(name="psB2t", bufs=1, space="PSUM")
        with pB2_ps as psB2, pB2_pt as psB2t:
            cast_i = 0

            def cast(out, in_):
                nonlocal cast_i
                cast_i += 1
                if cast_i % 2 == 0:
                    nc.vector.tensor_copy(out, in_)
                else:
                    nc.scalar.copy(out, in_)

            for g in range(G):
                # q: both heads' rows, token-chunks of 512 (+ 4-col tail)
                for c4 in range(4):
                    psq = psB2.tile([128, 512], F32, tag="psq", name="psq")
                    for ci in range(6):
                        nc.tensor.matmul(
                            psq[:], wqk[:, ci, g * 128:(g + 1) * 128],
                            xT[:, ci, c4 * 512:c4 * 512 + 512],
                            start=(ci == 0), stop=(ci == 5))
                    cast(qS[g][:, c4 * 512:c4 * 512 + 512], psq[:])
                psqt = psB2t.tile([128, 4], F32, tag="psqt", name="psqt")
                for ci in range(6):
                    nc.tensor.matmul(
                        psqt[:], wqk[:, ci, g * 128:(g + 1) * 128],
                        xT[:, ci, 2048:2052],
                        start=(ci == 0), stop=(ci == 5))
                cast(qS[g][:, 2048:2052], psqt[:])
                # k: per-batch 513-blocks
                for b in range(B_PER):
                    psk = psB2.tile([128, 512], F32, tag="psq", name="psk")
                    for ci in range(6):
                        nc.tensor.matmul(
                            psk[:], wqk[:, ci, DIM + g * 128:DIM + (g + 1) * 128],
                            xT[:, ci, b * N_TOK:b * N_TOK + 512],
                            start=(ci == 0), stop=(ci == 5))
                    cast(kS[g][:, b * N_TOK:b * N_TOK + 512], psk[:])
                psk4 = psB2t.tile([128, 4], F32, tag="psqt", name="psk4")
                for ci in range(6):
                    nc.tensor.matmul(
                        psk4[:], wqk[:, ci, DIM + g * 128:DIM + (g + 1) * 128],
                        xT.rearrange("p c (b n) -> p c b n", n=N_TOK)
                          [:, ci, :, 512],
                        start=(ci == 0), stop=(ci == 5))
                cast(kS[g][:, 0:T].rearrange(
                         "p (b w) -> p b w", w=N_TOK)[:, :, 512],
                     psk4[:])

        wqp_cm.__exit__(None, None, None)

        # ---------------- Phase B: v (per-batch key-tile layout) ---------
        pB_ps = tc.tile_pool(name="psB", bufs=3, space="PSUM")
        pB_pt = tc.tile_pool(name="psBt", bufs=1, space="PSUM")
        with pB_ps as psB, pB_pt as psBt:
            for b in range(B_PER):
                v3 = v8[b].rearrange("p m (h x) -> p m h x", x=HD + 1)
                for mt in range(4):
                    psv = psB.tile([128, DIM], F32, tag="psv", name="psv")
                    lhs0 = b * N_TOK + mt * 128
                    for ci in range(6):
                        lhsT = xT[:, ci, lhs0:lhs0 + 128]
                        nc.tensor.matmul(psv[:, 0:512], lhsT,
                                         wv[:, ci, 0:512],
                                         start=(ci == 0), stop=(ci == 5))
                        nc.tensor.matmul(psv[:, 512:768], lhsT,
                                         wv[:, ci, 512:768],
                                         start=(ci == 0), stop=(ci == 5))
                    src = psv[:].rearrange("p (h d) -> p h d", d=HD)
                    if b % 2 == 0:
                        nc.vector.tensor_copy(v3[:, mt, :, 0:HD], src)
                    else:
                        nc.scalar.copy(v3[:, mt, :, 0:HD], src)
            psvt = psBt.tile([128, DIM], F32, tag="psvt", name="psvt")
            for ci in range(6):
                lhsT = xT.rearrange("p c (b n) -> p c b n", n=N_TOK)[:, ci, :, 512]
                nc.tensor.matmul(psvt[:B_PER, 0:512], lhsT, wv[:, ci, 0:512],
                                 start=(ci == 0), stop=(ci == 5))
                nc.tensor.matmul(psvt[:B_PER, 512:768], lhsT, wv[:, ci, 512:768],
                                 start=(ci == 0), stop=(ci == 5))
            vst = pers.tile([B_PER, DIM], BF16, tag="vst", name="vst")
            nc.vector.tensor_copy(vst[:], psvt[:B_PER, :])
            for b in range(B_PER):
                v3 = v8[b].rearrange("p m (h x) -> p m h x", x=HD + 1)
                nc.sync.dma_start(
                    v3[0:1, 4, :, 0:HD],
                    vst.rearrange("p (h d) -> p h d", d=HD)[b:b + 1])

        wvp_cm.__exit__(None, None, None)
        xTp_cm.__exit__(None, None, None)

        # ---------------- Phase C + interleaved D ----------------
        aTp_cm = tc.tile_pool(name="aTp", bufs=1)
        aTp = aTp_cm.__enter__()
        aT = [aTp.tile([128, T], BF16, tag=f"aT{g}", name=f"aT{g}")
              for g in range(G)]
        wpal = aTp.tile([128, 6, DIM], BF16, tag="wpal", name="wpal")
        wp_src = wp_d.ap().rearrange("(c p) d -> p c d", p=128)
        for ci in range(6):
            nc.sync.dma_start(wpal[:, ci], wp_src[:, ci])
        pC_pm = tc.tile_pool(name="pmp", bufs=3)
        pC_pmb = tc.tile_pool(name="pmbp", bufs=4)
        pC_pt = tc.tile_pool(name="ptp", bufs=2)
        pC_sm = tc.tile_pool(name="smallp", bufs=2)
        pD3 = tc.tile_pool(name="ytp", bufs=2)
        pS_ps = tc.tile_pool(name="psS", bufs=2, space="PSUM")
        pU_ps = tc.tile_pool(name="psU", bufs=2, space="PSUM")
        pT_ps = tc.tile_pool(name="psT", bufs=1, space="PSUM")
        pD_ps = tc.tile_pool(name="psD", bufs=1, space="PSUM")
        d_done = [0]
        d_half = [0]
        d_yt = [None]
        d_avail = [0]
        with pC_pm as pmp, pC_pmb as pmbp, \
             pC_pt as ptp, pC_sm as smallp, pD3 as ytp, \
             pS_ps as psS, pU_ps as psU, pT_ps as psT, pD_ps as psD:

            def emit_d_pass():
                # one 6-matmul pass (half an output tile) per call; calls are
                # spaced by C work so the single psD buffer never stalls PE
                if d_done[0] >= len(t_tiles):
                    return False
                t0, ts = t_tiles[d_done[0]]
                if t0 + ts > d_avail[0]:
                    return False
                half = d_half[0]
                (c0, cw) = ((0, 512), (512, 256))[half]
                psy = psD.tile([128, 512], F32, tag="psy", name="psy")
                for j in range(6):
                    nc.tensor.matmul(psy[:ts, 0:cw], aT[j][:, t0:t0 + ts],
                                     wpal[:, j, c0:c0 + cw],
                                     start=(j == 0), stop=(j == 5))
                if half == 0:
                    yt = ytp.tile([128, DIM], F32, tag="yt", name="yt")
                    d_yt[0] = yt
                    nc.vector.tensor_add(yt[:ts, 0:512],
                                         psy[:ts, 0:512], bb[:ts, 0:512])
                    d_half[0] = 1
                else:
                    yt = d_yt[0]
                    nc.vector.tensor_add(yt[:ts, 512:768],
                                         psy[:ts, 0:256], bb[:ts, 512:768])
                    nc.sync.dma_start(y_d.ap()[t0:t0 + ts, :], yt[:ts, :])
                    d_half[0] = 0
                    d_done[0] += 1
                return True

            def emit_tail_av(st):
                # AV for the tail query of iteration st (ptail ready by now)
                b, g = st["b"], st["g"]
                for h2 in range(2):
                    h = 2 * g + h2
                    for mt in range(4):
                        nc.tensor.matmul(
                            st["stail"][0:HD + 1, 10 + h2:11 + h2],
                            v8[b][:, mt, h * 65:h * 65 + 65],
                            st["ptail"][:, 2 * mt + h2:2 * mt + h2 + 1],
                            start=(mt == 0), stop=False)
                    nc.tensor.matmul(
                        st["stail"][0:HD + 1, 10 + h2:11 + h2],
                        v8[b][0:1, 4, h * 65:h * 65 + 65],
                        st["ptail4"][0:1, h2:h2 + 1],
                        start=False, stop=True)

            def emit_norm_tail(st):
                b, g = st["b"], st["g"]
                rnt = smallp.tile([1, 2], F32, tag="rnt", name="rnt")
                nc.vector.reciprocal(rnt[:], st["stail"][HD:HD + 1, 10:12])
                for h2 in range(2):
                    bct = smallp.tile([HD, 1], F32, tag="bct", name="bct")
                    nc.gpsimd.partition_broadcast(bct[:], rnt[:, h2:h2 + 1])
                    nc.vector.tensor_mul(
                        aT[g][h2 * HD:h2 * HD + HD,
                              b * N_TOK + 512:b * N_TOK + 513],
                        st["stail"][0:HD, 10 + h2:11 + h2], bct[:])

            prev = [None]

            for b in range(B_PER):
                for g in range(G):
                    q, k = qS[g], kS[g]
                    q0 = b * N_TOK

                    def s_main(mt):
                        sm = psS.tile([128, 1024], F32, tag="S", name="S")
                        for h2 in range(2):
                            nc.tensor.matmul(
                                sm[:, h2 * 512:h2 * 512 + 512],
                                k[h2 * HD:h2 * HD + HD,
                                  q0 + mt * 128:q0 + mt * 128 + 128],
                                q[h2 * HD:h2 * HD + HD, q0:q0 + 512],
                                start=True, stop=True)
                        return sm

                    def exp_main(sm):
                        pm = pmp.tile([128, 1024], BF16, tag="pm", name="pm")
                        nc.scalar.activation(pm[:], sm[:], AF.Exp, bias=0.0,
                                             scale=ES)
                        return pm

                    def ebmul(pm, mt):
                        pmb = pmbp.tile([128, 1024], BF16, tag="pmb",
                                        name="pmb")
                        nc.vector.tensor_mul(pmb[:], pm[:], ebS[g][:, mt, :])
                        return pmb

                    def emit_av(mt, pmb):
                        for h2 in range(2):
                            h = 2 * g + h2
                            nc.tensor.matmul(
                                U2[h2][:],
                                v8[b][:, mt, h * 65:h * 65 + 65],
                                pmb[:, h2 * 512:h2 * 512 + 512],
                                start=(mt == 0), stop=False)

                    emit_d_pass()
                    U2 = [psU.tile([HD + 1, 512], F32, tag="U", name="U")
                          for _ in range(2)]
                    sm0 = s_main(0)
                    pm0 = exp_main(sm0)
                    sm1 = s_main(1)
                    if prev[0] is not None:
                        emit_tail_av(prev[0])
                        emit_norm_tail(prev[0])
                        if g == 0 and b > 0:
                            d_avail[0] = b * N_TOK
                    pm1 = exp_main(sm1)
                    pmb0 = ebmul(pm0, 0)
                    sm2 = s_main(2)
                    emit_av(0, pmb0)
                    pm2 = exp_main(sm2)
                    pmb1 = ebmul(pm1, 1)
                    # key-512 rank-1 row: S4 = k512^T q -> [1, 512] per h2
                    sm4 = psS.tile([128, 1024], F32, tag="S", name="S4")
                    for h2 in range(2):
                        nc.tensor.matmul(
                            sm4[0:1, h2 * 512:h2 * 512 + 512],
                            k[h2 * HD:h2 * HD + HD, q0 + 512:q0 + 513],
                            q[h2 * HD:h2 * HD + HD, q0:q0 + 512],
                            start=True, stop=True)
                    pm4 = pmp.tile([128, 1024], BF16, tag="pm", name="pm4")
                    nc.scalar.activation(pm4[0:1, :], sm4[0:1, :], AF.Exp,
                                         bias=0.0, scale=ES)
                    pmb4 = pmbp.tile([128, 1024], BF16, tag="pmb",
                                     name="pmb4")
                    nc.vector.tensor_mul(pmb4[0:1, :], pm4[0:1, :],
                                         eb4t[0:1, g * 1024:g * 1024 + 1024])
                    sm3 = s_main(3)
                    emit_av(1, pmb1)
                    pm3 = exp_main(sm3)
                    pmb2 = ebmul(pm2, 2)
                    # tail-query logits (consumed next iteration)
                    stail = psT.tile([128, 12], F32, tag="st", name="st")
                    for h2 in range(2):
                        for mt in range(4):
                            nc.tensor.matmul(
                                stail[:, 2 * mt + h2:2 * mt + h2 + 1],
                                k[h2 * HD:h2 * HD + HD,
                                  q0 + mt * 128:q0 + mt * 128 + 128],
                                q[h2 * HD:h2 * HD + HD, q0 + 512:q0 + 513],
                                start=True, stop=True)
                        nc.tensor.matmul(
                            stail[0:1, 8 + h2:9 + h2],
                            k[h2 * HD:h2 * HD + HD, q0 + 512:q0 + 513],
                            q[h2 * HD:h2 * HD + HD, q0 + 512:q0 + 513],
                            start=True, stop=True)
                    praw = smallp.tile([128, 8], F32, tag="praw", name="praw")
                    nc.scalar.activation(praw[:], stail[:, 0:8], AF.Exp,
                                         bias=0.0, scale=ES)
                    praw4 = smallp.tile([1, 2], F32, tag="praw4", name="praw4")
                    nc.scalar.activation(praw4[:], stail[0:1, 8:10], AF.Exp,
                                         bias=0.0, scale=ES)
                    ptail = ptp.tile([128, 8], BF16, tag="pt", name="pt")
                    nc.vector.tensor_mul(ptail[:], praw[:],
                                         ebt[:, g * 8:g * 8 + 8])
                    ptail4 = ptp.tile([1, 2], BF16, tag="pt4", name="pt4")
                    nc.vector.tensor_mul(ptail4[:], praw4[:],
                                         ebt[0:1, 48 + 2 * g:48 + 2 * g + 2])
                    emit_av(2, pmb2)
                    emit_d_pass()
                    pmb3 = ebmul(pm3, 3)
                    emit_av(3, pmb3)
                    # key-512 rank-1 AV: U += v512^T (x) pmb4 row, stop=True
                    for h2 in range(2):
                        h = 2 * g + h2
                        nc.tensor.matmul(
                            U2[h2][:],
                            v8[b][0:1, 4, h * 65:h * 65 + 65],
                            pmb4[0:1, h2 * 512:h2 * 512 + 512],
                            start=False, stop=True)
                    # normalize main block (DVE runs this during next iter's S)
                    for h2 in range(2):
                        rn = smallp.tile([1, 512], F32, tag="rn", name="rn")
                        nc.vector.reciprocal(rn[:], U2[h2][HD:HD + 1, :])
                        bc = smallp.tile([HD, 512], F32, tag="bc", name="bc")
                        nc.gpsimd.partition_broadcast(bc[:], rn[:])
                        nc.vector.tensor_mul(
                            aT[g][h2 * HD:h2 * HD + HD, q0:q0 + 512],
                            U2[h2][0:HD, :], bc[:])
                    prev[0] = {"b": b, "g": g, "stail": stail,
                               "ptail": ptail, "ptail4": ptail4}

            emit_tail_av(prev[0])
            emit_norm_tail(prev[0])
            d_avail[0] = T
            while emit_d_pass():
                pass

        aTp_cm.__exit__(None, None, None)
        pers_cm.__exit__(None, None, None)
        consts_cm.__exit__(None, None, None)

    nc.compile()
    return nc


def get_nc():
    if "nc" not in _CACHE:
        _CACHE["nc"] = _build_nc()
    return _CACHE["nc"]


def host_prep(w_qkv, bias_table, w_proj, b_proj, rel_index):
    """Host-side packing shared by all cores."""
    import ml_dtypes
    BF = ml_dtypes.bfloat16
    w = np.asarray(w_qkv, dtype=np.float32)
    wqkT = np.ascontiguousarray(w[0:2 * DIM].T).astype(BF)     # [c, 1536]
    wvT = np.ascontiguousarray(w[2 * DIM:3 * DIM].T).astype(BF)

    tbl = np.asarray(bias_table, dtype=np.float32)
    gat = tbl[np.asarray(rel_index)]            # [n(query), m(key), h]
    Bm = gat.transpose(2, 1, 0)                 # [h, m(key), n(query)]
    EBm = np.exp(Bm)                            # exp(bias), ~1 +- 0.1

    eb = np.zeros((G, 128, 4, 1024), dtype=np.float32)
    for g in range(G):
        for h2 in range(2):
            for mt in range(4):
                eb[g, :, mt, h2 * 512:h2 * 512 + 512] = \
                    EBm[2 * g + h2, mt * 128:mt * 128 + 128, 0:512]
    eb = eb.reshape(G, 128, 4096).astype(BF)

    eb4 = np.zeros((1, G * 1024), dtype=np.float32)
    for g in range(G):
        for h2 in range(2):
            eb4[0, g * 1024 + h2 * 512:g * 1024 + h2 * 512 + 512] = \
                EBm[2 * g + h2, 512, 0:512]
    eb4 = eb4.astype(BF)

    ebt = np.zeros((128, G * 8 + 16), dtype=np.float32)
    for g in range(G):
        for mt in range(4):
            for h2 in range(2):
                ebt[:, g * 8 + 2 * mt + h2] = \
                    EBm[2 * g + h2, mt * 128:mt * 128 + 128, 512]
        for h2 in range(2):
            ebt[0, 48 + 2 * g + h2] = EBm[2 * g + h2, 512, 512]

    wpT = np.ascontiguousarray(np.asarray(w_proj, dtype=np.float32).T).astype(BF)
    bb = np.ascontiguousarray(
        np.broadcast_to(np.asarray(b_proj, dtype=np.float32), (128, DIM)))
    return {"wqkT": wqkT, "wvT": wvT, "eb": eb, "eb4": eb4, "ebt": ebt,
            "wpT": wpT, "bb": bb}


def prep_x(x_core):
    """[B_PER, N_TOK, DIM] f32 -> xT [128, 6*T] bf16 (host transpose)."""
    import ml_dtypes
    xr = np.asarray(x_core, dtype=np.float32).reshape(T, 6, 128)
    return np.ascontiguousarray(
        xr.transpose(2, 1, 0).reshape(128, 6 * T)).astype(ml_dtypes.bfloat16)


def kernel(x, w_qkv, bias_table, w_proj, b_proj, rel_index):
    import time
    from concourse.bass_utils import run_bass_kernel_spmd

    x = np.asarray(x, dtype=np.float32)
    shared = host_prep(w_qkv, bias_table, w_proj, b_proj, rel_index)
    nc = get_nc()
    in_maps = []
    for c in range(N_CORES):
        m = {"xT": prep_x(x[c * B_PER:(c + 1) * B_PER])}
        m.update(shared)
        in_maps.append(m)
    # Transient NRT_EXEC_UNIT_UNRECOVERABLE failures have been observed on
    # this fabric; an identical retry passes, so guard the execution.
    last_exc = None
    for attempt in range(3):
        try:
            res = run_bass_kernel_spmd(nc, in_maps, core_ids=list(range(N_CORES)))
            break
        except Exception as e:
            last_exc = e
            time.sleep(2.0)
    else:
        raise last_exc
    out = np.concatenate(
        [res.results[c]["y"].reshape(B_PER, N_TOK, DIM) for c in range(N_CORES)],
        axis=0,
    )
    return out
